# revision 21
# baseline (speedup 1.0000x reference)
"""Trainium2 Bass kernel for BidPrefix: per-row cumprod + 3-point gather.

Reference semantics (per row b of inputs [B, 302]):
  rates = inputs[b, :300]; bid = int(inputs[b, 300]); mp = int(inputs[b, 301])
  cpz[k] = prod(rates[:k]) (cpz[0] = 1)
  out[b] = [cpz[bid], cpz[mp+1], cpz[mp]]

Strategy: pure data parallel over 8 NeuronCores (batch sharded, padded to
8*25088 rows). Per core, tiles of 128 rows grouped 28 at a time, with each
engine doing what it is natively fast at:

  DVE : ONE tensor_tensor_scan per tile (fp32 state) -> exact sequential
        f32 cumprod into a per-group [128, 28*301] cpz buffer.
  Pool: memset of the 28 cpz[0]=1 columns, idx cast to int16, and ONE
        ap_gather per group pulling all 28*3 taps per row from the cpz
        buffer (indices pre-offset by 301*tile via two iota base tables
        and three tiny int16 adds on DVE).
  DMA : streams tiles in; streams the raw [128, 1344] gather output to a
        DRAM scratch. ap_gather shares each index across the 16 channels
        of a Q7 core, so row r's taps sit at column (t*3+k)*16 + r%16 —
        a fixed skew undone for free on the host while unsharding.

Taps are read from the exact f32 cpz, so the result matches the f32
reference to ~1e-7.
"""

import sys

if "/opt/trn_rl_repo" not in sys.path:
    sys.path.insert(0, "/opt/trn_rl_repo")

import numpy as np

S = 300
SZ = S + 1  # 301 cpz entries per tile
COLS = 302
P = 128
NCORES = 8
TILES = 196
GROUP = 49
NG = TILES // GROUP
BPC = TILES * P  # 25088 rows per core
BTOT = 200000

TRACE = False
LAST_RESULTS = None


def build_nc(tiles=TILES, group=GROUP):
    import concourse.bacc as bacc
    import concourse.mybir as mybir
    from concourse import tile

    f32 = mybir.dt.float32
    f16 = mybir.dt.float16
    i16 = mybir.dt.int16
    A = mybir.AluOpType

    bpc = tiles * P
    if tiles % group != 0:
        group = tiles
    ngroups = tiles // group
    nidx = group * 3 * 16  # gathered elements per core-group of 16 rows

    nc = bacc.Bacc("TRN2", target_bir_lowering=False, debug=False)
    inp = nc.dram_tensor("inp", [bpc, COLS], f16, kind="ExternalInput")
    gout = nc.dram_tensor("gout", [ngroups, P, nidx], f32, kind="ExternalOutput")

    # row = p*tiles + t (partition-major)
    vin = inp.ap().rearrange("(p t) c -> p t c", p=P)

    with tile.TileContext(nc) as tc:
        with (
            tc.tile_pool(name="const", bufs=1) as constp,
            tc.tile_pool(name="raw", bufs=2) as rawp,
            tc.tile_pool(name="gath", bufs=1) as gathp,
            tc.tile_pool(name="grp", bufs=2) as grpp,
        ):
            # block base offsets 301*t (and +1 variant for the mp+1 tap)
            baseA = constp.tile([P, group], i16)
            nc.gpsimd.iota(baseA, pattern=[[SZ, group]], base=0, channel_multiplier=0)
            baseB = constp.tile([P, group], i16)
            nc.gpsimd.iota(baseB, pattern=[[SZ, group]], base=1, channel_multiplier=0)

            cpzbufs = []
            for b in range(2):
                cb = constp.tile([P, group, SZ], f32, tag=f"cpz{b}")
                nc.gpsimd.memset(cb[:, :, 0:1], 1.0)
                cpzbufs.append(cb)

            for g in range(ngroups):
                t0 = g * group
                braw = rawp.tile([P, group, COLS], f16, tag="braw")
                nc.sync.dma_start(braw, vin[:, t0 : t0 + group, :])

                cpz = cpzbufs[g % 2]
                for ti in range(group):
                    rates = braw[:, ti, 0:S]
                    nc.vector.tensor_tensor_scan(
                        cpz[:, ti, 1:SZ], rates, rates, 1.0, A.mult, A.bypass
                    )

                idx16 = grpp.tile([P, group, 2], i16, tag="idx16")
                nc.vector.tensor_copy(idx16, braw[:, :, S:COLS])
                idxs = grpp.tile([P, group, 3], i16, tag="idxs")
                nc.vector.tensor_tensor(idxs[:, :, 0], idx16[:, :, 0], baseA, A.add)
                nc.vector.tensor_tensor(idxs[:, :, 1], idx16[:, :, 1], baseB, A.add)
                nc.vector.tensor_tensor(idxs[:, :, 2], idx16[:, :, 1], baseA, A.add)

                gath = gathp.tile([P, nidx], f32, tag="gath")
                nc.gpsimd.ap_gather(
                    gath,
                    cpz.rearrange("p t z -> p (t z)"),
                    idxs.rearrange("p t k -> p (t k)"),
                    channels=P,
                    num_elems=group * SZ,
                    d=1,
                    num_idxs=nidx,
                )
                nc.scalar.dma_start(gout.ap()[g], gath)

    nc.compile()
    return nc


_NC_CACHE = {}


def _get_nc():
    key = (TILES, GROUP)
    if key not in _NC_CACHE:
        _NC_CACHE[key] = build_nc()
    return _NC_CACHE[key]


def deskew(go, tiles=TILES, group=GROUP):
    """[ngroups, P, group*3*16] skewed gather dump -> [P*tiles, 3] taps.

    ap_gather wraps each Q7 core's indices across its 16 partitions: row
    r's tap (t, k) value lands at column (t*3+k)*16 + r%16 of row r.
    """
    if tiles % group != 0:
        group = tiles
    ng = tiles // group
    v = go.reshape(ng, P, group * 3, 16)
    pm = (np.arange(P) % 16)[None, :, None, None]
    sel = np.take_along_axis(v, pm, axis=3)[..., 0]  # [ng, P, group*3]
    return (
        sel.transpose(1, 0, 2).reshape(P, tiles, 3).reshape(P * tiles, 3)
    )


def kernel(inputs):
    global LAST_RESULTS
    x = np.asarray(inputs).astype(np.float16)
    assert x.shape == (BTOT, COLS), x.shape

    npad = BPC * NCORES - BTOT
    padrows = np.zeros((npad, COLS), dtype=np.float16)
    padrows[:, :S] = 1.0
    xp = np.concatenate([x, padrows], axis=0)
    shards = xp.reshape(NCORES, BPC, COLS)

    in_maps = [{"inp": np.ascontiguousarray(shards[c])} for c in range(NCORES)]

    nc = _get_nc()
    from concourse.bass_utils import run_bass_kernel_spmd

    r = run_bass_kernel_spmd(
        nc, in_maps, core_ids=list(range(NCORES)), trace=TRACE
    )
    LAST_RESULTS = r
    y = np.concatenate(
        [deskew(np.asarray(r.results[c]["gout"])) for c in range(NCORES)], axis=0
    )
    return np.ascontiguousarray(y[:BTOT]).astype(np.float32)


# revision 22
# speedup vs baseline: 1.2354x; 1.2354x over previous
"""Trainium2 Bass kernel for BidPrefix: per-row cumprod + 3-point gather.

Reference semantics (per row b of inputs [B, 302]):
  rates = inputs[b, :300]; bid = int(inputs[b, 300]); mp = int(inputs[b, 301])
  cpz[k] = prod(rates[:k]) (cpz[0] = 1)
  out[b] = [cpz[bid], cpz[mp+1], cpz[mp]]

Strategy: pure data parallel over 8 NeuronCores (batch sharded, padded to
8*25088 rows). Per core, tiles of 128 rows grouped 28 at a time, with each
engine doing what it is natively fast at:

  DVE : ONE tensor_tensor_scan per tile (fp32 state) -> exact sequential
        f32 cumprod into a per-group [128, 28*301] cpz buffer.
  Pool: memset of the 28 cpz[0]=1 columns, idx cast to int16, and ONE
        ap_gather per group pulling all 28*3 taps per row from the cpz
        buffer (indices pre-offset by 301*tile via two iota base tables
        and three tiny int16 adds on DVE).
  DMA : streams tiles in; streams the raw [128, 1344] gather output to a
        DRAM scratch. ap_gather shares each index across the 16 channels
        of a Q7 core, so row r's taps sit at column (t*3+k)*16 + r%16 —
        a fixed skew undone for free on the host while unsharding.

Taps are read from the exact f32 cpz, so the result matches the f32
reference to ~1e-7.
"""

import sys

if "/opt/trn_rl_repo" not in sys.path:
    sys.path.insert(0, "/opt/trn_rl_repo")

import numpy as np

S = 300
SZ = S + 1  # 301 cpz entries per tile
COLS = 302
P = 128
NCORES = 8
TILES = 196
GROUP = 14
NG = TILES // GROUP
BPC = TILES * P  # 25088 rows per core
BTOT = 200000

TRACE = False
LAST_RESULTS = None


def build_nc(tiles=TILES, group=GROUP):
    import concourse.bacc as bacc
    import concourse.mybir as mybir
    from concourse import tile

    f32 = mybir.dt.float32
    f16 = mybir.dt.float16
    i16 = mybir.dt.int16
    A = mybir.AluOpType

    bpc = tiles * P
    if tiles % group != 0:
        group = tiles
    ngroups = tiles // group
    nidx = group * 3 * 16  # gathered elements per core-group of 16 rows

    nc = bacc.Bacc("TRN2", target_bir_lowering=False, debug=False)
    inp = nc.dram_tensor("inp", [bpc, COLS], f16, kind="ExternalInput")
    gout = nc.dram_tensor("gout", [ngroups, P, nidx], f32, kind="ExternalOutput")

    # row = p*tiles + t (partition-major)
    vin = inp.ap().rearrange("(p t) c -> p t c", p=P)

    with tile.TileContext(nc) as tc:
        with (
            tc.tile_pool(name="const", bufs=1) as constp,
            tc.tile_pool(name="raw", bufs=4) as rawp,
            tc.tile_pool(name="gath", bufs=3) as gathp,
            tc.tile_pool(name="grp", bufs=2) as grpp,
        ):
            # block base offsets 301*t (and +1 variant for the mp+1 tap)
            baseA = constp.tile([P, group], i16)
            nc.gpsimd.iota(baseA, pattern=[[SZ, group]], base=0, channel_multiplier=0)
            baseB = constp.tile([P, group], i16)
            nc.gpsimd.iota(baseB, pattern=[[SZ, group]], base=1, channel_multiplier=0)

            cpzbufs = []
            for b in range(4):
                cb = constp.tile([P, group, SZ], f32, tag=f"cpz{b}")
                nc.gpsimd.memset(cb[:, :, 0:1], 1.0)
                cpzbufs.append(cb)

            for g in range(ngroups):
                t0 = g * group
                braw = rawp.tile([P, group, COLS], f16, tag="braw")
                nc.sync.dma_start(braw, vin[:, t0 : t0 + group, :])

                cpz = cpzbufs[g % 4]
                for ti in range(group):
                    rates = braw[:, ti, 0:S]
                    nc.vector.tensor_tensor_scan(
                        cpz[:, ti, 1:SZ], rates, rates, 1.0, A.mult, A.bypass
                    )

                idx16 = grpp.tile([P, group, 2], i16, tag="idx16")
                nc.vector.tensor_copy(idx16, braw[:, :, S:COLS])
                idxs = grpp.tile([P, group, 3], i16, tag="idxs")
                nc.vector.tensor_tensor(idxs[:, :, 0], idx16[:, :, 0], baseA, A.add)
                nc.vector.tensor_tensor(idxs[:, :, 1], idx16[:, :, 1], baseB, A.add)
                nc.vector.tensor_tensor(idxs[:, :, 2], idx16[:, :, 1], baseA, A.add)

                gath = gathp.tile([P, nidx], f32, tag="gath")
                nc.gpsimd.ap_gather(
                    gath,
                    cpz.rearrange("p t z -> p (t z)"),
                    idxs.rearrange("p t k -> p (t k)"),
                    channels=P,
                    num_elems=group * SZ,
                    d=1,
                    num_idxs=nidx,
                )
                nc.scalar.dma_start(gout.ap()[g], gath)

    nc.compile()
    return nc


_NC_CACHE = {}


def _get_nc():
    key = (TILES, GROUP)
    if key not in _NC_CACHE:
        _NC_CACHE[key] = build_nc()
    return _NC_CACHE[key]


def deskew(go, tiles=TILES, group=GROUP):
    """[ngroups, P, group*3*16] skewed gather dump -> [P*tiles, 3] taps.

    ap_gather wraps each Q7 core's indices across its 16 partitions: row
    r's tap (t, k) value lands at column (t*3+k)*16 + r%16 of row r.
    """
    if tiles % group != 0:
        group = tiles
    ng = tiles // group
    v = go.reshape(ng, P, group * 3, 16)
    pm = (np.arange(P) % 16)[None, :, None, None]
    sel = np.take_along_axis(v, pm, axis=3)[..., 0]  # [ng, P, group*3]
    return (
        sel.transpose(1, 0, 2).reshape(P, tiles, 3).reshape(P * tiles, 3)
    )


def kernel(inputs):
    global LAST_RESULTS
    x = np.asarray(inputs).astype(np.float16)
    assert x.shape == (BTOT, COLS), x.shape

    npad = BPC * NCORES - BTOT
    padrows = np.zeros((npad, COLS), dtype=np.float16)
    padrows[:, :S] = 1.0
    xp = np.concatenate([x, padrows], axis=0)
    shards = xp.reshape(NCORES, BPC, COLS)

    in_maps = [{"inp": np.ascontiguousarray(shards[c])} for c in range(NCORES)]

    nc = _get_nc()
    from concourse.bass_utils import run_bass_kernel_spmd

    r = run_bass_kernel_spmd(
        nc, in_maps, core_ids=list(range(NCORES)), trace=TRACE
    )
    LAST_RESULTS = r
    y = np.concatenate(
        [deskew(np.asarray(r.results[c]["gout"])) for c in range(NCORES)], axis=0
    )
    return np.ascontiguousarray(y[:BTOT]).astype(np.float32)
